# revision 1
# baseline (speedup 1.0000x reference)
"""Gaussian blur 101x101 (separable) on 4096x4096 fp32, 8 NeuronCores.

Strategy: the 2D conv kernel W = outer(gv, gh) is rank-1, so the blur is two
1D 101-tap convs. Rows are sharded 512/core; each core gets a host-prepared
padded strip (50-row halo, zero-padded edges, plus 50/78 zero columns) so the
on-device program is uniform across cores with no collectives.

Each 1D conv maps onto the PE array as banded matmuls with 128-row
contraction windows:
  pass1: tmpT[j', i] = sum_r x[r, j'] gv[r - i + 50]
         matmul(lhsT = x[rows win, cols 128a:+128], rhs = Gv_d) -> PSUM
  pass2: out[i, j] = sum_j' tmpT[j', i] gh[j' - j + 50]
         matmul(lhsT = tmpT[win a][:, 128c:+128], rhs = Gh_d) -> PSUM
with shared band tiles G_d[k, f] = g[k - f + d], d in {0, 128, 256},
f-chunks of 256 (float32r runs 1 cycle/row at moving dim >= 256).
tmpT tiles are stored at the 128-row windows pass2 needs (offset -50), so no
transposes or partition-shifts are required anywhere.
"""

import os
import time
from contextlib import ExitStack

import numpy as np

import concourse.bass as bass  # noqa: F401  (AP types come via tile/bacc)
import concourse.mybir as mybir
import concourse.tile as tile
from concourse import bacc, bass_utils

H = 4096
W = 4096
TAPS = 101
PAD = 50
N_CORES = 8
RPC = H // N_CORES          # 512 output rows per core
NW1 = 5                     # input row windows of 128 per core
XP_ROWS = 128 * NW1         # 640 = 512 + 100 halo + 28 slack (zeros)
NA = 33                     # tmpT column windows of 128
XP_COLS = 128 * NA          # 4224 = 50 + 4096 + 78 (cols incl zero pads)
FB = 256                    # band free width per matmul
DT = mybir.dt.float32

_compiled = {}


class _FastExitTC(tile.TileContext):
    """TileContext whose exit skips the per-semaphore clear storm.

    The stock exit emits dma_reset + sem_clear for every allocated semaphore
    (~250 here) plus a second all-engine barrier — ~8us of pure tail on a
    NEFF that is loaded, executed once, and unloaded. The drain + one
    barrier (which gate output-DMA completion) are kept.
    """

    def _drain_and_barrier(self, tick_clock, wait_clock):
        from concourse.vector_clock import ScopedClock

        drain_inst = self.nc.sync.drain()
        wait_clock.add_sem_waits(
            drain_inst.ins, ScopedClock({None: tick_clock.global_clock})
        )
        self.nc.all_engine_barrier()
        popped = self.nc._tile_sem_poison_stack.pop()
        assert popped is self._sem_poison


def _build_nc(mm_dtype):
    nc = bacc.Bacc(
        "TRN2",
        target_bir_lowering=False,
        debug=False,
        enable_asserts=False,
        num_devices=N_CORES,
    )
    xp = nc.dram_tensor("xp", [XP_ROWS, XP_COLS], mm_dtype, kind="ExternalInput").ap()
    bandsV = nc.dram_tensor(
        "bandsV", [128, 3 * FB], mm_dtype, kind="ExternalInput"
    ).ap()
    bandsH = nc.dram_tensor(
        "bandsH", [128, 3 * FB], mm_dtype, kind="ExternalInput"
    ).ap()
    y = nc.dram_tensor("y", [RPC, W], DT, kind="ExternalOutput").ap()

    with _FastExitTC(nc) as tc, ExitStack() as ctx:
        xw_pool = ctx.enter_context(tc.tile_pool(name="xw", bufs=1))
        band_pool = ctx.enter_context(tc.tile_pool(name="bands", bufs=1))
        tm_pool = ctx.enter_context(tc.tile_pool(name="tm", bufs=1))
        p1_pool = ctx.enter_context(tc.tile_pool(name="p1", bufs=4, space="PSUM"))
        p2_pool = ctx.enter_context(tc.tile_pool(name="p2", bufs=4, space="PSUM"))
        st_pool = ctx.enter_context(tc.tile_pool(name="st", bufs=6))

        # column-chunked window loads so pass1's first tiles aren't gated on
        # full 2.2MB window transfers; chunk order matches pass1's a-order
        ccuts = [0, 256, 640, 1280, 2304, 3328, XP_COLS]
        xw = [
            xw_pool.tile([128, XP_COLS], mm_dtype, tag=f"xw{w}", name=f"xw{w}")
            for w in range(NW1)
        ]

        # spread DMA issue over two HWDGE queues — a single queue only issues
        # one descriptor-gen op per ~600ns, which starves the PE at kernel start
        dma_engines = [nc.sync, nc.scalar]
        # PE warmup: fp32 matmuls on a DVE-memset scratch tile need no DMA,
        # so they start ~4us in and HAM reaches K=8/8 before real data lands.
        # The warmup psum shares the p2 pool's slots (released before pass 2).
        wt = band_pool.tile([128, FB], DT, tag="wt", name="wt")
        nc.vector.memset(wt[:], 0.0)
        wps = p2_pool.tile([128, FB], DT, name="wps", tag="ps2")
        for _ in range(8):
            nc.tensor.matmul(
                wps[:], lhsT=wt[:, 0:128], rhs=wt[:], start=True, stop=True
            )

        bv = band_pool.tile([128, 3 * FB], mm_dtype, tag="bv")
        nc.sync.dma_start(bv[:], bandsV[:])
        bh = band_pool.tile([128, 3 * FB], mm_dtype, tag="bh")
        nc.scalar.dma_start(bh[:], bandsH[:])
        k = 0
        for ci in range(len(ccuts) - 1):
            cs, ce = ccuts[ci], ccuts[ci + 1]
            for w in range(NW1):
                eng = dma_engines[k % 2]
                k += 1
                eng.dma_start(xw[w][:, cs:ce], xp[128 * w : 128 * (w + 1), cs:ce])

        # pass 1 and pass 2 interleaved in emission order: pass2 group t2
        # needs tm windows up to a = 4*t2 + 4, so it is emitted right after
        # that pass1 tile. The static PE schedule then backfills pass2
        # matmuls into pass1's input-DMA stalls, and output DMA overlaps
        # input DMA instead of forming a burst at the end.
        def pass2_group(t2):
            for cpt in range(RPC // 128):
                ps2 = p2_pool.tile([128, 2 * FB], DT, tag="ps2", name=f"ps2_{t2}_{cpt}")
                for hf in range(2):
                    b2 = 2 * t2 + hf
                    for ai in range(3):
                        a2 = 2 * b2 + ai
                        nc.tensor.matmul(
                            ps2[:, FB * hf : FB * (hf + 1)],
                            lhsT=tm[a2][:, 128 * cpt : 128 * (cpt + 1)],
                            rhs=bh[:, FB * ai : FB * (ai + 1)],
                            start=(ai == 0),
                            stop=(ai == 2),
                        )
                st = st_pool.tile([128, 2 * FB], DT, name=f"st_{t2}_{cpt}", tag="st")
                nc.scalar.copy(st[:], ps2[:])
                eng = dma_engines[(t2 * 4 + cpt) % 2]
                eng.dma_start(
                    y[128 * cpt : 128 * (cpt + 1), 512 * t2 : 512 * (t2 + 1)],
                    st[:],
                )

        tm = []
        for a in range(NA):
            ps1 = p1_pool.tile([128, RPC], DT, tag="ps1", name=f"ps1_{a}")
            for b in range(2):
                for di in range(3):
                    w = 2 * b + di
                    nc.tensor.matmul(
                        ps1[:, FB * b : FB * (b + 1)],
                        lhsT=xw[w][:, 128 * a : 128 * (a + 1)],
                        rhs=bv[:, FB * di : FB * (di + 1)],
                        start=(di == 0),
                        stop=(di == 2),
                    )
            tma = tm_pool.tile([128, RPC], mm_dtype, tag=f"tm{a}", name=f"tm{a}")
            nc.vector.tensor_copy(tma[:], ps1[:])
            tm.append(tma)
            if a >= 4 and a % 4 == 0:
                pass2_group(a // 4 - 1)

    nc.compile()
    return nc


def _get_nc(mm_dtype):
    key = str(mm_dtype)
    if key not in _compiled:
        _compiled[key] = _build_nc(mm_dtype)
    return _compiled[key]


def _make_band(g, d):
    # G_d[k, f] = g[k - f + d], zero outside [0, TAPS)
    idx = np.arange(128)[:, None] - np.arange(FB)[None, :] + d
    valid = (idx >= 0) & (idx < TAPS)
    return np.where(valid, g[np.clip(idx, 0, TAPS - 1)], 0.0).astype(np.float32)


def kernel(x: np.ndarray, weight: np.ndarray) -> np.ndarray:
    x = np.asarray(x, dtype=np.float32)
    Wm = np.asarray(weight, dtype=np.float32).reshape(TAPS, TAPS)
    assert x.shape == (H, W), x.shape

    # rank-1 (separable) decomposition of the 2D kernel
    u, s, vt = np.linalg.svd(Wm.astype(np.float64))
    gv = (u[:, 0] * np.sqrt(s[0]))
    gh = (vt[0] * np.sqrt(s[0]))
    if gv.sum() < 0:
        gv, gh = -gv, -gh
    gv = gv.astype(np.float32)
    gh = gh.astype(np.float32)

    bandsV = np.concatenate([_make_band(gv, d) for d in (0, 128, 256)], axis=1)
    bandsH = np.concatenate([_make_band(gh, d) for d in (0, 128, 256)], axis=1)

    # padded per-core strips: rows [r0-50, r0+590), cols [-50, 4174), zeros
    # outside the image
    in_maps = []
    for c in range(N_CORES):
        r0 = c * RPC
        xp = np.zeros((XP_ROWS, XP_COLS), np.float32)
        lo = r0 - PAD
        hi = min(r0 + RPC + PAD, H)
        src_lo = max(lo, 0)
        xp[src_lo - lo : hi - lo, PAD : PAD + W] = x[src_lo:hi]
        in_maps.append({"xp": xp, "bandsV": bandsV, "bandsH": bandsH})

    mm_dtype = (
        mybir.dt.float32
        if os.environ.get("BLUR_MM_DTYPE") == "fp32"
        else mybir.dt.float32r
    )
    nc = _get_nc(mm_dtype)

    trace = os.environ.get("BLUR_TRACE") == "1"
    res = None
    last_exc = None
    for attempt in range(3):
        try:
            res = bass_utils.run_bass_kernel_spmd(
                nc, in_maps, core_ids=list(range(N_CORES)), trace=trace
            )
            break
        except Exception as e:  # transient NRT/device blips — retry
            last_exc = e
            time.sleep(2.0)
    if res is None:
        raise last_exc
    if trace:
        print(f"HW exec time: {res.exec_time_ns} ns")
        print(f"mean exec time: {res.mean_exec_time_ns} ns")
        if res.instructions_and_trace is not None:
            print(f"trace: {res.instructions_and_trace[1]}")

    out = np.concatenate([res.results[c]["y"] for c in range(N_CORES)], axis=0)
    return out[None, None]



# revision 4
# speedup vs baseline: 1.4354x; 1.4354x over previous
"""Gaussian blur 101x101 (separable) on 4096x4096 fp32, 8 NeuronCores.

v2: bf16 everywhere + band-stationary pass 2 + transposed packed output.

The 2D kernel W = outer(gv, gh) is rank-1, so the blur is two 1D 101-tap
convs. Rows are sharded 512/core; each core gets a host-prepared padded
strip (50-row halo, zero-padded edges) in bf16 (tolerance is 2e-2; bf16
adds ~0.1% L2 error but halves DMA bytes vs fp32).

Pass 1 (vertical conv, x-stationary): for each 128-col block a and each
128-row output chunk rc, two accumulating matmuls
    tmpT[c in a, r'] += xw[rc][:, a]^T @ Gv_0  +  xw[rc+1][:, a]^T @ Gv_1
with band tiles Gv_d[p, q] = gv[p - q + d], d in {0, 128}. N=128 moving
keeps it at 2 matmuls per 128 output cols (vs 3 per 256 at N=256).

Pass 2 (horizontal conv, BAND-stationary): the Gaussian band is the PE
weight, tmpT tiles stream as rhs with N=512:
    yT[c' in n, r'] = Gh_0^T @ tmpT[n]  +  Gh_1^T @ tmpT[n+1]
Only 2 matmuls per 128 output cols at N=512 (vs 3 per 256 for the
data-stationary form). The output lands transposed (yT[c', r']); four
consecutive 128-row blocks are packed side by side in SBUF and written
with a single DMA of 4KB contiguous lines. The host un-transposes for
free after gather.
"""

import os
import time
from contextlib import ExitStack

import ml_dtypes
import numpy as np

import concourse.bass as bass  # noqa: F401  (AP types come via tile/bacc)
import concourse.mybir as mybir
import concourse.tile as tile
from concourse import bacc, bass_utils

H = 4096
W = 4096
TAPS = 101
PAD = 50
N_CORES = 8
RPC = H // N_CORES          # 512 output rows per core
NW1 = 5                     # input row windows of 128 per core
XP_ROWS = 128 * NW1         # 640 = 512 + 100 halo + 28 slack (zeros)
NA = 33                     # tmpT column windows of 128
XP_COLS = 128 * NA          # 4224 = 50 + 4096 + 78 (cols incl zero pads)
NB = 32                     # output column blocks of 128
DT = mybir.dt.float32
BF = mybir.dt.bfloat16

_compiled = {}


class _FastExitTC(tile.TileContext):
    """TileContext whose exit skips the per-semaphore clear storm.

    The stock exit emits dma_reset + sem_clear for every allocated semaphore
    plus a second all-engine barrier — ~8us of pure tail on a NEFF that is
    loaded, executed once, and unloaded. The drain + one barrier (which gate
    output-DMA completion) are kept.
    """

    def _drain_and_barrier(self, tick_clock, wait_clock):
        from concourse.vector_clock import ScopedClock

        drain_inst = self.nc.sync.drain()
        wait_clock.add_sem_waits(
            drain_inst.ins, ScopedClock({None: tick_clock.global_clock})
        )
        self.nc.all_engine_barrier()
        popped = self.nc._tile_sem_poison_stack.pop()
        assert popped is self._sem_poison

def _build_nc():
    nc = bacc.Bacc(
        "TRN2",
        target_bir_lowering=False,
        debug=False,
        enable_asserts=False,
        num_devices=N_CORES,
    )
    xp = nc.dram_tensor("xp", [XP_ROWS, XP_COLS], BF, kind="ExternalInput").ap()
    bandsV = nc.dram_tensor("bandsV", [128, 256], BF, kind="ExternalInput").ap()
    bandsH = nc.dram_tensor("bandsH", [128, 256], BF, kind="ExternalInput").ap()
    # packed transposed output: row 128p+q, col 512k+f  <->  yT[128(4p+k)+q, f]
    y = nc.dram_tensor("y", [128 * (NB // 4), 4 * RPC], BF, kind="ExternalOutput").ap()

    with _FastExitTC(nc) as tc, ExitStack() as ctx:
        xw_pool = ctx.enter_context(tc.tile_pool(name="xw", bufs=1))
        band_pool = ctx.enter_context(tc.tile_pool(name="bands", bufs=1))
        tm_pool = ctx.enter_context(tc.tile_pool(name="tm", bufs=1))
        p1_pool = ctx.enter_context(tc.tile_pool(name="p1", bufs=4, space="PSUM"))
        p2_pool = ctx.enter_context(tc.tile_pool(name="p2", bufs=4, space="PSUM"))
        st_pool = ctx.enter_context(tc.tile_pool(name="st", bufs=3))

        xw = [
            xw_pool.tile([128, XP_COLS], BF, tag=f"xw{w}", name=f"xw{w}")
            for w in range(NW1)
        ]

        # spread DMA issue over two HWDGE queues — a single queue only issues
        # one descriptor-gen op per ~600ns
        dma_engines = [nc.sync, nc.scalar]
        # PE warmup: matmuls on a DVE-memset scratch tile need no DMA, so
        # they start early and the PE p-state ramps before real data lands.
        wt = band_pool.tile([128, 512], BF, tag="wt", name="wt")
        nc.vector.memset(wt[:], 0.0)
        wps = p2_pool.tile([128, 512], DT, name="wps", tag="ps2")
        for _ in range(8):
            nc.tensor.matmul(
                wps[:], lhsT=wt[:, 0:128], rhs=wt[:], start=True, stop=True
            )

        bv = band_pool.tile([128, 256], BF, tag="bv")
        nc.sync.dma_start(bv[:], bandsV[:])
        bh = band_pool.tile([128, 256], BF, tag="bh")
        nc.scalar.dma_start(bh[:], bandsH[:])

        # column-chunked window loads so pass1's first tiles aren't gated on
        # full window transfers; chunk order matches pass1's a-order
        ccuts = [0, 512, 1536, 2560, 3584, XP_COLS]
        k = 0
        for ci in range(len(ccuts) - 1):
            cs, ce = ccuts[ci], ccuts[ci + 1]
            for w in range(NW1):
                eng = dma_engines[k % 2]
                k += 1
                eng.dma_start(xw[w][:, cs:ce], xp[128 * w : 128 * (w + 1), cs:ce])

        st = [None] * (NB // 4)
        out_engines = [nc.sync, nc.sync]

        def emit_pass2(n):
            p = n // 4
            if n % 4 == 0:
                st[p] = st_pool.tile([128, 2048], BF, tag="st", name=f"st{p}")
            ps2 = p2_pool.tile([128, 512], DT, tag="ps2", name=f"ps2_{n}")
            nc.tensor.matmul(
                ps2[:], lhsT=bh[:, 0:128], rhs=tm[n][:], start=True, stop=False
            )
            nc.tensor.matmul(
                ps2[:], lhsT=bh[:, 128:256], rhs=tm[n + 1][:], start=False, stop=True
            )
            q = n % 4
            nc.vector.tensor_copy(st[p][:, 512 * q : 512 * (q + 1)], ps2[:])
            if q == 3:
                eng = out_engines[p % 2]
                eng.dma_start(y[128 * p : 128 * (p + 1), :], st[p][:])

        # pass 1 and pass 2 interleaved in emission order so the static PE
        # schedule backfills pass2 matmuls into pass1's input-DMA stalls and
        # output DMA overlaps input DMA
        tm = []
        for a in range(NA):
            ps1 = p1_pool.tile([128, 512], DT, tag="ps1", name=f"ps1_{a}")
            for rc in range(4):
                nc.tensor.matmul(
                    ps1[:, 128 * rc : 128 * (rc + 1)],
                    lhsT=xw[rc][:, 128 * a : 128 * (a + 1)],
                    rhs=bv[:, 0:128],
                    start=True,
                    stop=False,
                )
                nc.tensor.matmul(
                    ps1[:, 128 * rc : 128 * (rc + 1)],
                    lhsT=xw[rc + 1][:, 128 * a : 128 * (a + 1)],
                    rhs=bv[:, 128:256],
                    start=False,
                    stop=True,
                )
            tma = tm_pool.tile([128, 512], BF, tag=f"tm{a}", name=f"tm{a}")
            nc.scalar.copy(tma[:], ps1[:])
            tm.append(tma)
            if a >= 1:
                emit_pass2(a - 1)

    nc.compile()
    return nc


def _get_nc():
    if "nc" not in _compiled:
        _compiled["nc"] = _build_nc()
    return _compiled["nc"]


def _make_band(g, d, fb=128):
    # G_d[k, f] = g[k - f + d], zero outside [0, TAPS)
    idx = np.arange(128)[:, None] - np.arange(fb)[None, :] + d
    valid = (idx >= 0) & (idx < TAPS)
    return np.where(valid, g[np.clip(idx, 0, TAPS - 1)], 0.0).astype(np.float32)


def kernel(x: np.ndarray, weight: np.ndarray) -> np.ndarray:
    x = np.asarray(x, dtype=np.float32)
    Wm = np.asarray(weight, dtype=np.float32).reshape(TAPS, TAPS)
    assert x.shape == (H, W), x.shape

    # rank-1 (separable) decomposition of the 2D kernel
    u, s, vt = np.linalg.svd(Wm.astype(np.float64))
    gv = (u[:, 0] * np.sqrt(s[0]))
    gh = (vt[0] * np.sqrt(s[0]))
    if gv.sum() < 0:
        gv, gh = -gv, -gh
    gv = gv.astype(np.float32)
    gh = gh.astype(np.float32)

    bandsV = np.concatenate(
        [_make_band(gv, 0), _make_band(gv, 128)], axis=1
    ).astype(ml_dtypes.bfloat16)
    bandsH = np.concatenate(
        [_make_band(gh, 0), _make_band(gh, 128)], axis=1
    ).astype(ml_dtypes.bfloat16)

    # padded per-core strips: rows [r0-50, r0+590), cols [-50, 4174), zeros
    # outside the image
    in_maps = []
    for c in range(N_CORES):
        r0 = c * RPC
        xp = np.zeros((XP_ROWS, XP_COLS), np.float32)
        lo = r0 - PAD
        hi = min(r0 + RPC + PAD, H)
        src_lo = max(lo, 0)
        xp[src_lo - lo : hi - lo, PAD : PAD + W] = x[src_lo:hi]
        in_maps.append(
            {
                "xp": xp.astype(ml_dtypes.bfloat16),
                "bandsV": bandsV,
                "bandsH": bandsH,
            }
        )

    nc = _get_nc()

    trace = os.environ.get("BLUR_TRACE") == "1"
    res = None
    last_exc = None
    for attempt in range(3):
        try:
            res = bass_utils.run_bass_kernel_spmd(
                nc, in_maps, core_ids=list(range(N_CORES)), trace=trace
            )
            break
        except Exception as e:  # transient NRT/device blips — retry
            last_exc = e
            time.sleep(2.0)
    if res is None:
        raise last_exc
    if trace:
        print(f"HW exec time: {res.exec_time_ns} ns")
        print(f"mean exec time: {res.mean_exec_time_ns} ns")
        if res.instructions_and_trace is not None:
            print(f"trace: {res.instructions_and_trace[1]}")

    # unpack: y[128p+q, 512k+f] = yT[128(4p+k)+q, f]; strip = yT^T
    strips = []
    for c in range(N_CORES):
        yp = np.asarray(res.results[c]["y"]).astype(np.float32)
        yT = (
            yp.reshape(NB // 4, 128, 4, 512)
            .transpose(0, 2, 1, 3)
            .reshape(W, RPC)
        )
        strips.append(yT.T)
    out = np.concatenate(strips, axis=0)
    return out[None, None]
